# revision 23
# baseline (speedup 1.0000x reference)
"""AdditiveAttention kernel for one TRN2 chip (8 NeuronCores), Bass/Tile.

Reference (per batch b):
    Q = query @ Wq.T            # [S, S]
    K = key_  @ Wk.T            # [S, S]
    scores = tanh(Q + K)
    V = value @ Wv.T            # [S, D]
    scores[mask] = -1e10
    attn = softmax(scores, -1)  # [S, S]
    ctx = attn @ V              # [S, D]
    out = ctx @ Wfc.T + value   # [S, D]
    return (out, attn)

Shapes: B=4, S=4096, D=512.

Sharding: 8 shards = (batch b, seq-half h); each core owns 2048 query rows of
one batch. All weights replicated. Host pre-transposes + casts the matmul
operands (layout prep only; all FLOPs of the module run on device).

Precision strategy (rel-err budget 2e-2; measured ~2.5e-3):
  - scores matmul (QK) in bf16: feeds softmax directly, needs the accuracy.
  - everything downstream of the softmax numerator runs fp8e4 DoubleRow on
    the PE (attn@V, V-proj, FC): these only touch `out` through ctx, which is
    ~30x smaller than the residual, so fp8 noise is invisible there.
  - attn output = bf16 p * f32 recip -> f32.

Per s-tile of 128 query rows:
  scores chunk [128,1024] = PSUM accum of bf16 matmuls (q + k d-blocks)
  ACT: tanh (PSUM->SBUF bf16), exp (->bf16)
  DVE: p = exp * notm with fused row-sum (tensor_tensor_reduce)
  DMA: transpose p chunk (xbar) -> bf16, DVE cast -> fp8 pairs
  PE : ctx += pT.T @ V as fp8 DoubleRow (2 K-blocks per instr)
  after t loop: recip = 1/rowsum; attn = p * recip (ACT, ->f32, DMA out);
  ctx evac with recip scale; DMA-transpose ctx; FC fp8 DoubleRow + residual.
"""

import os
import sys

sys.path.insert(0, "/opt/trn_rl_repo")

import numpy as np
import ml_dtypes

BF = ml_dtypes.bfloat16
F8 = ml_dtypes.float8_e4m3

B, S, D = 4, 4096, 512
SQ = 2048          # query rows per core
N_CORES = 8
NT = SQ // 128     # 16 s-tiles per core
DB = D // 128      # 4 d-blocks

# fp8 DoubleRow for the post-softmax matmuls (attn@V, V-proj, FC).
USE_FP8 = os.environ.get("KERNEL_FP8", "1") == "1"


def build_bass(repeat=1):
    import concourse.bass as bass
    import concourse.mybir as mybir
    import concourse.tile as tile
    from concourse import bacc
    from concourse.masks import make_identity

    f32 = mybir.dt.float32
    bf16 = mybir.dt.bfloat16
    fp8 = mybir.dt.float8e4 if USE_FP8 else mybir.dt.bfloat16
    DR = mybir.MatmulPerfMode.DoubleRow if USE_FP8 else None

    # Bacc (not plain Bass): its finalize() runs generate_event_semaphores,
    # which splits >1 sync-waits per instruction into EventSemaphore pairs —
    # walrus codegen rejects multi-wait DMA/matmul structs otherwise.
    nc = bacc.Bacc()

    if repeat != 1:
        # unique unused param so the PJRT-level HLO (and its compile cache
        # key) differs between repeat variants
        nc.declare_dram_parameter("rep_tag", [1, repeat], f32, isOutput=False)
    qT = nc.declare_dram_parameter("qT", [D, SQ], bf16, isOutput=False)
    kT = nc.declare_dram_parameter("kT", [D, SQ], bf16, isOutput=False)
    vT = nc.declare_dram_parameter("vT", [D, S], fp8, isOutput=False)
    vres = nc.declare_dram_parameter("vres", [SQ, D], f32, isOutput=False)
    notm = nc.declare_dram_parameter("notm", [SQ, S], bf16, isOutput=False)
    wqT = nc.declare_dram_parameter("wqT", [D, S], bf16, isOutput=False)
    wkT = nc.declare_dram_parameter("wkT", [D, S], bf16, isOutput=False)
    wvT = nc.declare_dram_parameter("wvT", [D, D], fp8, isOutput=False)
    wfcT = nc.declare_dram_parameter("wfcT", [D, D], fp8, isOutput=False)
    attn_d = nc.declare_dram_parameter("attn", [SQ, S], f32, isOutput=True)
    out_d = nc.declare_dram_parameter("out", [SQ, D], f32, isOutput=True)

    CH = 1024               # post-matmul pipeline chunk width
    NCH = S // CH           # 4 chunks per row
    JB = CH // 128          # 8 t-subblocks per chunk

    def pairs(ap2d, j2):
        """[128, k] AP slice -> [128, 2, 128] DoubleRow pair view."""
        return ap2d[:, j2 * 256:(j2 + 1) * 256].rearrange(
            "p (two m) -> p two m", two=2
        )

    with tile.TileContext(nc) as tc:
        with (
            tc.tile_pool(name="singles", bufs=1) as singles,
            tc.tile_pool(name="p_pool", bufs=2) as p_pool,
            tc.tile_pool(name="notm_pool", bufs=3) as notm_pool,
            tc.tile_pool(name="chunks", bufs=3) as chunks,
            tc.tile_pool(name="attn_pool", bufs=2) as attn_pool,
            tc.tile_pool(name="stile", bufs=2) as stile,
            tc.tile_pool(name="psum_sc", bufs=2, space="PSUM") as psum_sc,
            tc.tile_pool(name="psum_tr", bufs=2, space="PSUM") as psum_tr,
            tc.tile_pool(name="psum_cf", bufs=2, space="PSUM") as psum_cf,
        ):
            ident = singles.tile([128, 128], bf16)
            make_identity(nc, ident)
            # V-proj inputs first so PE can start ASAP
            wv_sb = singles.tile([128, DB, D], fp8)
            nc.sync.dma_start(out=wv_sb, in_=wvT.rearrange("(a p) e -> p a e", p=128))
            vT_sb = singles.tile([128, DB, S], fp8)
            vT_r = vT.rearrange("(a p) t -> p a t", p=128)
            for c4 in range(4):
                t_lo = c4 * (S // 4)
                nc.sync.dma_start(
                    out=vT_sb[:, :, t_lo:t_lo + S // 4],
                    in_=vT_r[:, :, t_lo:t_lo + S // 4],
                )
            # full qT/kT upfront: decouples the s-loop from per-tile loads
            qT_sb = singles.tile([128, DB, SQ], bf16)
            nc.sync.dma_start(out=qT_sb, in_=qT.rearrange("(a p) s -> p a s", p=128))
            kT_sb = singles.tile([128, DB, SQ], bf16)
            nc.sync.dma_start(out=kT_sb, in_=kT.rearrange("(a p) s -> p a s", p=128))
            wfc_sb = singles.tile([128, DB, D], fp8)
            nc.sync.dma_start(out=wfc_sb, in_=wfcT.rearrange("(a p) e -> p a e", p=128))
            # QK weights split into t-pieces so early score chunks can start
            wq_sb = singles.tile([128, DB, S], bf16)
            wk_sb = singles.tile([128, DB, S], bf16)
            wq_r = wqT.rearrange("(a p) t -> p a t", p=128)
            wk_r = wkT.rearrange("(a p) t -> p a t", p=128)
            for c4 in range(NCH):
                t_lo = c4 * CH
                nc.sync.dma_start(
                    out=wq_sb[:, :, t_lo:t_lo + CH], in_=wq_r[:, :, t_lo:t_lo + CH]
                )
                nc.sync.dma_start(
                    out=wk_sb[:, :, t_lo:t_lo + CH], in_=wk_r[:, :, t_lo:t_lo + CH]
                )
            V_sb = singles.tile([128, S // 128, D], fp8)

            # ---- V projection: V[t, d] = value @ Wv.T (fp8 DoubleRow) ----
            for t in range(S // 128):
                v_ps = psum_sc.tile([128, D], f32, tag="sc")
                if USE_FP8:
                    for a2 in range(2):
                        nc.tensor.matmul(
                            v_ps,
                            lhsT=vT_sb[:, 2 * a2:2 * a2 + 2, t * 128:(t + 1) * 128],
                            rhs=wv_sb[:, 2 * a2:2 * a2 + 2, :],
                            start=(a2 == 0),
                            stop=(a2 == 1),
                            perf_mode=DR,
                        )
                else:
                    for a in range(DB):
                        nc.tensor.matmul(
                            v_ps,
                            lhsT=vT_sb[:, a, t * 128:(t + 1) * 128],
                            rhs=wv_sb[:, a, :],
                            start=(a == 0),
                            stop=(a == DB - 1),
                        )
                nc.scalar.copy(V_sb[:, t, :], v_ps)

            # ---- main loop over s-tiles ----
            for i in [i for _ in range(repeat) for i in range(NT)]:
                s_lo = i * 128
                qT_t = qT_sb[:, :, s_lo:s_lo + 128]
                kT_t = kT_sb[:, :, s_lo:s_lo + 128]

                p_bf = p_pool.tile([128, S], bf16, tag="p")
                denom_parts = stile.tile([128, NCH], f32, tag="dparts")
                ctx_ps = psum_cf.tile([128, D], f32, tag="cf")

                for c in range(NCH):
                    t_lo = c * CH
                    sc_ps = psum_sc.tile([128, CH], f32, tag="sc")
                    for h in range(CH // 512):
                        hs = h * 512
                        for a in range(DB):
                            nc.tensor.matmul(
                                sc_ps[:, hs:hs + 512],
                                lhsT=qT_t[:, a, :],
                                rhs=wq_sb[:, a, t_lo + hs:t_lo + hs + 512],
                                start=(a == 0),
                                stop=False,
                            )
                        for a in range(DB):
                            nc.tensor.matmul(
                                sc_ps[:, hs:hs + 512],
                                lhsT=kT_t[:, a, :],
                                rhs=wk_sb[:, a, t_lo + hs:t_lo + hs + 512],
                                start=False,
                                stop=(a == DB - 1),
                            )
                    tanh_bf = chunks.tile([128, CH], bf16, tag="tanh")
                    nc.scalar.activation(
                        tanh_bf, sc_ps, mybir.ActivationFunctionType.Tanh
                    )
                    exp_bf = chunks.tile([128, CH], bf16, tag="exp")
                    nc.scalar.activation(
                        exp_bf, tanh_bf, mybir.ActivationFunctionType.Exp
                    )
                    notm_c = notm_pool.tile([128, CH], bf16, tag="notm")
                    nc.sync.dma_start(
                        out=notm_c, in_=notm[s_lo:s_lo + 128, t_lo:t_lo + CH]
                    )
                    # p = exp * (1 - mask), then row-sum into denom_parts[:, c]
                    # (tensor_tensor_reduce would fuse these, but that custom
                    # DVE opcode hard-crashes the exec unit on this runtime)
                    nc.vector.tensor_mul(
                        out=p_bf[:, t_lo:t_lo + CH], in0=exp_bf, in1=notm_c
                    )
                    nc.vector.reduce_sum(
                        denom_parts[:, c:c + 1],
                        p_bf[:, t_lo:t_lo + CH],
                        axis=mybir.AxisListType.X,
                    )
                    # transpose p chunk on PE -> psum bf16, evac+cast -> fp8
                    pt_ps = psum_tr.tile([128, CH], bf16, tag="tr")
                    for j in range(JB):
                        nc.tensor.transpose(
                            pt_ps[:, j * 128:(j + 1) * 128],
                            p_bf[:, t_lo + j * 128:t_lo + (j + 1) * 128],
                            ident,
                        )
                    pt8 = chunks.tile([128, CH], fp8, tag="pt8")
                    nc.vector.tensor_copy(out=pt8, in_=pt_ps)
                    # ctx += pT.T @ V  (fp8 DoubleRow, 2 t-blocks per instr)
                    if USE_FP8:
                        for j2 in range(JB // 2):
                            tb = c * JB + 2 * j2
                            nc.tensor.matmul(
                                ctx_ps,
                                lhsT=pairs(pt8, j2),
                                rhs=V_sb[:, tb:tb + 2, :],
                                start=(c == 0 and j2 == 0),
                                stop=(c == NCH - 1 and j2 == JB // 2 - 1),
                                perf_mode=DR,
                            )
                    else:
                        for j in range(JB):
                            nc.tensor.matmul(
                                ctx_ps,
                                lhsT=pt8[:, j * 128:(j + 1) * 128],
                                rhs=V_sb[:, c * JB + j, :],
                                start=(c == 0 and j == 0),
                                stop=(c == NCH - 1 and j == JB - 1),
                            )

                # ---- row stats ----
                denom = stile.tile([128, 1], f32, tag="denom")
                nc.vector.reduce_sum(denom, denom_parts, axis=mybir.AxisListType.X)
                recip = stile.tile([128, 1], f32, tag="recip")
                nc.vector.reciprocal(recip, denom)

                # ---- attn = p * recip -> DRAM (f32) ----
                for c in range(NCH):
                    t_lo = c * CH
                    attn_st = attn_pool.tile([128, CH], f32, tag="attn_st")
                    nc.scalar.mul(attn_st, p_bf[:, t_lo:t_lo + CH], recip[:, 0:1])
                    nc.sync.dma_start(
                        out=attn_d[s_lo:s_lo + 128, t_lo:t_lo + CH],
                        in_=attn_st,
                    )

                # ---- ctx evac (normalize-after-matmul), transpose, FC ----
                ctx_sb = stile.tile([128, D], bf16, tag="ctx_sb")
                nc.scalar.mul(ctx_sb, ctx_ps, recip[:, 0:1])
                ctxT_ps = psum_tr.tile([128, CH], bf16, tag="tr")
                for j in range(DB):
                    nc.tensor.transpose(
                        ctxT_ps[:, j * 128:(j + 1) * 128],
                        ctx_sb[:, j * 128:(j + 1) * 128],
                        ident,
                    )
                ctxT8 = stile.tile([128, D], fp8, tag="ctxT8")
                nc.vector.tensor_copy(out=ctxT8, in_=ctxT_ps[:, :D])
                fc_ps = psum_cf.tile([128, D], f32, tag="cf")
                if USE_FP8:
                    for a2 in range(2):
                        nc.tensor.matmul(
                            fc_ps,
                            lhsT=pairs(ctxT8, a2),
                            rhs=wfc_sb[:, 2 * a2:2 * a2 + 2, :],
                            start=(a2 == 0),
                            stop=(a2 == 1),
                            perf_mode=DR,
                        )
                else:
                    for j in range(DB):
                        nc.tensor.matmul(
                            fc_ps,
                            lhsT=ctxT8[:, j * 128:(j + 1) * 128],
                            rhs=wfc_sb[:, j, :],
                            start=(j == 0),
                            stop=(j == DB - 1),
                        )
                vres_sb = stile.tile([128, D], f32, tag="vres_sb")
                nc.sync.dma_start(out=vres_sb, in_=vres[s_lo:s_lo + 128, :])
                out_sb = stile.tile([128, D], f32, tag="out_sb")
                nc.vector.tensor_add(out=out_sb, in0=fc_ps, in1=vres_sb)
                nc.sync.dma_start(out=out_d[s_lo:s_lo + 128, :], in_=out_sb)

    # Bacc defers register allocation to finalize()/compile(); the SPMD run
    # path serializes nc.m directly, so finalize here.
    nc.finalize()
    return nc


_CACHED_NC = None


def kernel(query, key_, value, mask, Wq, Wk, Wv, Wfc, _profile=False):
    global _CACHED_NC
    from concourse.bass_utils import run_bass_kernel_spmd

    F8x = F8 if USE_FP8 else BF
    q = np.asarray(query, dtype=np.float32)
    k = np.asarray(key_, dtype=np.float32)
    v = np.asarray(value, dtype=np.float32)
    m = np.asarray(mask)
    wqT = np.ascontiguousarray(np.asarray(Wq, np.float32).T).astype(BF)
    wkT = np.ascontiguousarray(np.asarray(Wk, np.float32).T).astype(BF)
    wvT = np.ascontiguousarray(np.asarray(Wv, np.float32).T).astype(F8x)
    wfcT = np.ascontiguousarray(np.asarray(Wfc, np.float32).T).astype(F8x)

    in_maps = []
    for c in range(N_CORES):
        b, h = divmod(c, 2)
        s0 = h * SQ
        in_maps.append({
            "qT": np.ascontiguousarray(q[b, s0:s0 + SQ].T).astype(BF),
            "kT": np.ascontiguousarray(k[b, s0:s0 + SQ].T).astype(BF),
            "vT": np.ascontiguousarray(v[b].T).astype(F8x),
            "vres": np.ascontiguousarray(v[b, s0:s0 + SQ]),
            "notm": (~m[b, s0:s0 + SQ]).astype(BF),
            "wqT": wqT,
            "wkT": wkT,
            "wvT": wvT,
            "wfcT": wfcT,
        })

    if _CACHED_NC is None:
        _CACHED_NC = build_bass()
    res = run_bass_kernel_spmd(
        _CACHED_NC, in_maps, list(range(N_CORES)), trace=_profile
    )

    out = np.empty((B, S, D), np.float32)
    attn = np.empty((B, S, S), np.float32)
    for c in range(N_CORES):
        b, h = divmod(c, 2)
        s0 = h * SQ
        out[b, s0:s0 + SQ] = res.results[c]["out"]
        attn[b, s0:s0 + SQ] = res.results[c]["attn"]

    if _profile:
        kernel.last_exec_time_ns = res.exec_time_ns
        kernel.last_results = res
    return out, attn
